# revision 28
# baseline (speedup 1.0000x reference)
"""Trainium2 Bass kernel for an 8-batch single-head attention block.

Reference computation (per batch b of 8, S=2048 seq, D=A=768):
    Q = relu(X Wq + bq); K = relu(X Wk + bk); V = relu(X Wv + bv)
    P = softmax(Q K^T)          (no 1/sqrt(d) scale)
    X1 = LN(X + P V)
    X2 = LN(X1 + X1 Wd + bd)    (LN affines are identity in this problem)

Sharding: data-parallel - batch b -> NeuronCore b (8 cores, no collectives).

v2 design notes (from the v1 trace, 480 us):
  * Every matmul lowers to LDWEIGHTS+MATMUL; fp32r LDWEIGHTS is 224 ns
    (two passes) vs 120 ns for bf16, and it binds the PE issue rate for
    free dims < ~448.  All matmul operands are therefore bf16 (PSUM
    accumulation and LN arithmetic stay fp32).  Measured N=512 MM-to-MM
    spacing is ~259 ns either way, so bf16 costs nothing on the streams.
  * Q^T stays resident in SBUF (bf16, 1.5 MB) - no DRAM spill/reload.
  * Weight DMAs ride the gpsimd queue so the scalar engine stream is
    pure compute; wk + xt[0] issue first so the PE starts ~2 us in
    (v1 idled 24 us at the start waiting on one big weight blob).
  * The scalar engine runs only Relu/Exp/Sqrt.  Affines/squares run on
    the DVE, and the staggered tail keeps each chunk's sqrts contiguous
    in the scalar stream, so table-set switches are ~2 per chunk.
    (exp(-0.5*ln(var+eps)) was tried to get a zero-switch schedule but
    walrus maps Ln to a set without exp: 16 switches/chunk, worse.)
  * Phase C per q-chunk: scores (16k x 6e MMs) -> exp -> per-qs attn
    (PSUM col 768 of V==1.0 gives softmax row sums) -> LN1 on DVE ->
    bf16 X1 -> PE transpose (bf16, 1 cyc/row) -> dense proj -> LN2 ->
    out rows.  The per-qs tail is emitted one qs behind the attn groups
    so LN latency hides under the next attn matmul group.
  * PSUM: pst 2 + pa0 2 + pa1 1 + pt 1 + pp0 1 + pp1 1 = 8 banks.
"""

from contextlib import ExitStack

import numpy as np
import ml_dtypes

import concourse.bass as bass
import concourse.mybir as mybir
import concourse.tile as tile
from concourse import bacc
from concourse.bass_utils import run_bass_kernel_spmd
from concourse.masks import make_identity

S, D = 2048, 768
N_CORES = 8
SB, DB = S // 128, D // 128  # 16 s-blocks, 6 d-blocks
SCH = 512   # phase-B s-chunk width
QCH = 512   # phase-C q-chunk width
NCH = S // SCH
F32 = mybir.dt.float32
BF16 = mybir.dt.bfloat16
AF = mybir.ActivationFunctionType
ALU = mybir.AluOpType
EPS = 1e-5
BF16NP = ml_dtypes.bfloat16


def _split_matmul_waits(nc):
    """Walrus allows only one semaphore wait on self-loading (fp32/fp32r/
    transpose) Matmult instructions; move extra waits onto a preceding
    InstEventSemaphore (which may carry two waits each)."""
    for bb in nc.main_func.blocks:
        new_insts = []
        for inst in bb.instructions:
            if isinstance(inst, mybir.InstMatmult) and inst.sync_info is not None \
                    and len(inst.sync_info.on_wait) > 1:
                waits = list(inst.sync_info.on_wait)
                extra, keep = waits[:-1], waits[-1:]
                while extra:
                    chunk, extra = extra[:2], extra[2:]
                    ev = mybir.InstEventSemaphore(
                        name=nc.get_next_instruction_name(), ins=[], outs=[])
                    ev.engine = inst.engine
                    ev.sync_info = mybir.SyncInfo(on_wait=chunk, on_update=[])
                    nc.register_instruction(ev)
                    new_insts.append(ev)
                inst.sync_info = mybir.SyncInfo(
                    on_wait=keep, on_update=list(inst.sync_info.on_update))
            new_insts.append(inst)
        bb.instructions[:] = new_insts


def _build():
    nc = bacc.Bacc("TRN2", target_bir_lowering=False, debug=False,
                   enable_asserts=False, num_devices=N_CORES)

    x_d = nc.dram_tensor("x", [S, D], F32, kind="ExternalInput").ap()
    xt_d = nc.dram_tensor("xt", [DB, 128, S], BF16, kind="ExternalInput").ap()
    wq_d = nc.dram_tensor("wq", [DB, 128, D], BF16, kind="ExternalInput").ap()
    wk_d = nc.dram_tensor("wk", [DB, 128, D], BF16, kind="ExternalInput").ap()
    wv_d = nc.dram_tensor("wv", [DB, 128, D + 2], BF16, kind="ExternalInput").ap()
    wd_d = nc.dram_tensor("wd", [DB, 128, D], BF16, kind="ExternalInput").ap()
    bqk_d = nc.dram_tensor("bqk", [128, 2 * DB], F32, kind="ExternalInput").ap()
    bv_d = nc.dram_tensor("bv", [128, D + 2], F32, kind="ExternalInput").ap()
    bd_d = nc.dram_tensor("bd", [128, D], F32, kind="ExternalInput").ap()
    out_d = nc.dram_tensor("out", [S, D], F32, kind="ExternalOutput").ap()

    with tile.TileContext(nc) as tc, ExitStack() as ctx:
        consts = ctx.enter_context(tc.tile_pool(name="consts", bufs=1))
        pers = ctx.enter_context(tc.tile_pool(name="pers", bufs=1))
        wdp = ctx.enter_context(tc.tile_pool(name="wdp", bufs=1))

        # bqk first on the sync queue (tiny - K-relu biases needed early)
        bqk_sb = consts.tile([128, 2 * DB], F32, tag="bqk", name="bqk")
        nc.sync.dma_start(bqk_sb[:], bqk_d[:])
        bq_sb = [bqk_sb[:, e:e + 1] for e in range(DB)]
        bk_sb = [bqk_sb[:, DB + e:DB + e + 1] for e in range(DB)]

        magic = consts.tile([128, 1], mybir.dt.int32, tag="magic", name="magic")
        nc.gpsimd.memset(magic[:], 0x5F3759DF)
        one_i = consts.tile([128, 1], mybir.dt.int32, tag="one_i", name="one_i")
        nc.gpsimd.memset(one_i[:], 1)

        # persistent bf16 operand tiles
        kt = {}
        qt = {}
        v_sb = []
        for k in range(SB):
            v_sb.append(pers.tile([128, D + 2], BF16, tag=f"v{k}", name=f"v{k}"))

        # ---------------- Phase B: K^T, Q^T, V (all resident, bf16)
        with tc.tile_pool(name="wqkv", bufs=1) as wpool, \
             tc.tile_pool(name="xtp", bufs=1) as xtp, \
             tc.tile_pool(name="bvb", bufs=2) as bvb, \
             tc.tile_pool(name="bpm", bufs=4, space="PSUM") as bpm:
            # Startup DMA spread: wk on the sync queue and xt chunk 0 on the
            # scalar queue (both idle at start) so the first K matmul group
            # can start ~5us in; everything else rides the gpsimd queue.
            wk_sb, wq_sb, wv_sb = [], [], []
            for d in range(DB):
                t = wpool.tile([128, D], BF16, tag=f"wk{d}", name=f"wk{d}")
                nc.sync.dma_start(t[:], wk_d[d])
                wk_sb.append(t)
            # per-chunk xt tiles: no shared-tile write deps, so the first K
            # matmul group waits only on wk + xt[*][0]
            xt_sb = {}
            for d in range(DB):
                for c in range(NCH):
                    xt_sb[(d, c)] = xtp.tile([128, SCH], BF16,
                                             tag=f"xt{d}_{c}",
                                             name=f"xt{d}_{c}")
            for d in range(DB):
                nc.scalar.dma_start(xt_sb[(d, 0)][:], xt_d[d, :, 0:SCH])
            for c in range(1, NCH):
                for d in range(DB):
                    nc.sync.dma_start(xt_sb[(d, c)][:],
                                      xt_d[d, :, c * SCH:(c + 1) * SCH])
            ident = consts.tile([128, 128], BF16, tag="ident", name="ident")
            make_identity(nc, ident[:])
            eps_sb = consts.tile([128, 1], F32, tag="eps", name="eps")
            nc.gpsimd.memset(eps_sb[:], EPS)
            bv_sb = consts.tile([128, D + 2], F32, tag="bv", name="bv")
            nc.gpsimd.dma_start(bv_sb[:], bv_d[:])
            for d in range(DB):
                t = wpool.tile([128, D + 2], BF16, tag=f"wv{d}", name=f"wv{d}")
                nc.gpsimd.dma_start(t[:], wv_d[d])
                wv_sb.append(t)
            for d in range(DB):
                t = wpool.tile([128, D], BF16, tag=f"wq{d}", name=f"wq{d}")
                nc.gpsimd.dma_start(t[:], wq_d[d])
                wq_sb.append(t)
            wd_sb = []
            for d in range(DB):
                t = wdp.tile([128, D], BF16, tag=f"wd{d}", name=f"wd{d}")
                nc.gpsimd.dma_start(t[:], wd_d[d])
                wd_sb.append(t)
            bd_sb = consts.tile([128, D], F32, tag="bd", name="bd")
            nc.gpsimd.dma_start(bd_sb[:], bd_d[:])

            nsb = SCH // 128  # s-blocks per chunk
            for c in range(NCH):
                for e in range(DB):
                    pk = bpm.tile([128, SCH], F32, tag="pmm", name="pmm")
                    for d in range(DB):
                        nc.tensor.matmul(pk[:], wk_sb[d][:, e * 128:(e + 1) * 128],
                                         xt_sb[(d, c)][:],
                                         start=(d == 0), stop=(d == DB - 1))
                    kt_t = pers.tile([128, SCH], BF16, tag=f"kt{e}_{c}",
                                     name=f"kt{e}_{c}")
                    nc.scalar.activation(kt_t[:], pk[:], AF.Relu, bias=bk_sb[e])
                    kt[(e, c)] = kt_t
                # V s-blocks (col 768 == 1.0 via bv_aug for softmax row-sums)
                for sb in range(nsb):
                    k_idx = c * nsb + sb
                    for n0, nw in ((0, 512), (512, D + 2 - 512)):
                        pv = bpm.tile([128, 512], F32, tag="pmm", name="pmm")
                        for d in range(DB):
                            nc.tensor.matmul(pv[:, :nw],
                                             xt_sb[(d, c)][:, sb * 128:
                                                           (sb + 1) * 128],
                                             wv_sb[d][:, n0:n0 + nw],
                                             start=(d == 0), stop=(d == DB - 1))
                        vb = bvb.tile([128, 512], F32, tag="vb", name="vb")
                        nc.vector.tensor_add(vb[:, :nw], pv[:, :nw],
                                             bv_sb[:, n0:n0 + nw])
                        nc.scalar.activation(v_sb[k_idx][:, n0:n0 + nw],
                                             vb[:, :nw], AF.Relu)
                for e in range(DB):
                    pq = bpm.tile([128, SCH], F32, tag="pmm", name="pmm")
                    for d in range(DB):
                        nc.tensor.matmul(pq[:], wq_sb[d][:, e * 128:(e + 1) * 128],
                                         xt_sb[(d, c)][:],
                                         start=(d == 0), stop=(d == DB - 1))
                    qt_t = pers.tile([128, SCH], BF16, tag=f"qt{e}_{c}",
                                     name=f"qt{e}_{c}")
                    nc.scalar.activation(qt_t[:], pq[:], AF.Relu, bias=bq_sb[e])
                    qt[(e, c)] = qt_t

        # ------- Phase C (fused): scores -> exp -> attn -> LN1 -> proj -> LN2
        with tc.tile_pool(name="cx", bufs=2) as cx, \
             tc.tile_pool(name="cxr", bufs=1) as cxr, \
             tc.tile_pool(name="cx1", bufs=1) as cx1, \
             tc.tile_pool(name="cet", bufs=2) as cet, \
             tc.tile_pool(name="cst", bufs=2, space="PSUM") as cst, \
             tc.tile_pool(name="cpa0", bufs=2, space="PSUM") as cpa0, \
             tc.tile_pool(name="cpa1", bufs=2, space="PSUM") as cpa1, \
             tc.tile_pool(name="cpp0", bufs=1, space="PSUM") as cpp0, \
             tc.tile_pool(name="cpp1", bufs=1, space="PSUM") as cpp1:
            nqb = QCH // 128  # q-blocks per chunk
            kt_per_chunk = SCH // 128

            def ln_stats(prefix, src, accs, use_sqrt):
                """negmu/rstd from partial row-sums `accs` and full row
                `src`.  The square+row-sum runs on the scalar engine
                (Square is in every ACT table set - no table switch).
                rstd: DVE Quake-rsqrt while later exps are still coming
                (a scalar Sqrt would thrash table sets against them);
                plain scalar Sqrt in the last chunk where no exps follow."""
                negmu = cx.tile([128, 1], F32, tag=f"{prefix}nm", name=f"{prefix}nm")
                nc.vector.tensor_add(negmu[:], accs[0][:], accs[1][:])
                nc.vector.tensor_scalar(negmu[:], negmu[:], -1.0 / D, None,
                                        op0=ALU.mult)
                sq = cx.tile([128, D], F32, tag=f"{prefix}sq", name=f"{prefix}sq",
                             bufs=1)
                ssq = cx.tile([128, 1], F32, tag=f"{prefix}ssq", name=f"{prefix}ssq")
                nc.scalar.activation(sq[:], src[:], AF.Square,
                                     accum_out=ssq[:])
                mu2e = cx.tile([128, 1], F32, tag=f"{prefix}mu2", name=f"{prefix}mu2")
                nc.vector.scalar_tensor_tensor(
                    mu2e[:], negmu[:], negmu[:], eps_sb[:],
                    op0=ALU.mult, op1=ALU.subtract)
                var = cx.tile([128, 1], F32, tag=f"{prefix}var", name=f"{prefix}var")
                nc.vector.scalar_tensor_tensor(
                    var[:], ssq[:], 1.0 / D, mu2e[:],
                    op0=ALU.mult, op1=ALU.subtract)  # = true var + eps
                rstd = cx.tile([128, 1], F32, tag=f"{prefix}rs", name=f"{prefix}rs")
                if use_sqrt:
                    sd = cx.tile([128, 1], F32, tag=f"{prefix}sd",
                                 name=f"{prefix}sd")
                    nc.scalar.activation(sd[:], var[:], AF.Sqrt)
                    nc.vector.reciprocal(rstd[:], sd[:])
                else:
                    yi = cx.tile([128, 1], mybir.dt.int32, tag=f"{prefix}yi",
                                 name=f"{prefix}yi")
                    nc.vector.tensor_scalar(yi[:],
                                            var[:].bitcast(mybir.dt.int32),
                                            one_i[:], None,
                                            op0=ALU.arith_shift_right)
                    nc.vector.tensor_tensor(yi[:], magic[:], yi[:],
                                            op=ALU.subtract)
                    y = yi[:].bitcast(F32)
                    for it in range(2):
                        w = cx.tile([128, 1], F32, tag=f"{prefix}w{it}",
                                    name=f"{prefix}w{it}")
                        nc.vector.scalar_tensor_tensor(
                            w[:], y, y, var[:], op0=ALU.mult, op1=ALU.mult)
                        s = cx.tile([128, 1], F32, tag=f"{prefix}s{it}",
                                    name=f"{prefix}s{it}")
                        nc.vector.tensor_scalar(s[:], w[:], -0.5, 1.5,
                                                op0=ALU.mult, op1=ALU.add)
                        dst = rstd if it == 1 else cx.tile(
                            [128, 1], F32, tag=f"{prefix}y{it}",
                            name=f"{prefix}y{it}")
                        nc.vector.tensor_mul(dst[:], s[:], y)
                        y = dst[:]
                nmr = cx.tile([128, 1], F32, tag=f"{prefix}nmr", name=f"{prefix}nmr")
                nc.vector.tensor_mul(nmr[:], negmu[:], rstd[:])
                return rstd, nmr

            x_res = {}
            x1_t = {}
            x1t_t = {}

            def tail(c, qs, use_sqrt=False):
                """transpose X1[qs] -> X1^T, dense proj, LN2, out rows."""
                x1 = x1_t[qs]
                # pt shares the score pool's tag/slots (bf16 768 fits a
                # 512-f32 slot) - PSUM stays at 8 banks
                pt = cst.tile([128, D], BF16, tag="pst", name="pt")
                for d in range(DB):
                    nc.tensor.transpose(
                        pt[:, d * 128:(d + 1) * 128],
                        x1[:, d * 128:(d + 1) * 128], ident[:])
                x1t = cx1.tile([128, D], BF16, tag=f"x1t{qs}", name=f"x1t{qs}")
                nc.vector.tensor_copy(x1t[:], pt[:])
                x1t_t[qs] = x1t
                pp0 = cpp0.tile([128, 512], F32, tag="pp0", name="pp0")
                for d in range(DB):
                    nc.tensor.matmul(pp0[:], x1t[:, d * 128:(d + 1) * 128],
                                     wd_sb[d][:, 0:512],
                                     start=(d == 0), stop=(d == DB - 1))
                pp1 = cpp1.tile([128, 256], F32, tag="pp1", name="pp1")
                for d in range(DB):
                    nc.tensor.matmul(pp1[:], x1t[:, d * 128:(d + 1) * 128],
                                     wd_sb[d][:, 512:D],
                                     start=(d == 0), stop=(d == DB - 1))
                x1bd = cx.tile([128, D], F32, tag="x1bd", name="x1bd")
                nc.vector.tensor_add(x1bd[:], x1[:], bd_sb[:])
                y_t = cx.tile([128, D], F32, tag="y_t", name="y_t")
                t0 = cx.tile([128, 1], F32, tag="t0", name="t0")
                t1 = cx.tile([128, 1], F32, tag="t1", name="t1")
                nc.vector.scalar_tensor_tensor(
                    y_t[:, 0:512], pp0[:], 0.0, x1bd[:, 0:512],
                    op0=ALU.add, op1=ALU.add, accum_out=t0[:])
                nc.vector.scalar_tensor_tensor(
                    y_t[:, 512:D], pp1[:], 0.0, x1bd[:, 512:D],
                    op0=ALU.add, op1=ALU.add, accum_out=t1[:])
                rstd2, nmr2 = ln_stats("l2", y_t, (t0, t1), use_sqrt)
                out_t = cx.tile([128, D], F32, tag="out_t", name="out_t")
                nc.vector.tensor_scalar(out_t[:], y_t[:], rstd2[:], nmr2[:],
                                        op0=ALU.mult, op1=ALU.add)
                r0 = c * QCH + qs * 128
                nc.sync.dma_start(out_d[r0:r0 + 128, :], out_t[:])

            pending_tail = None
            for c in range(NCH):
                # residual rows for this chunk (sync queue; arrives well
                # before LN1 needs it)
                for qs in range(nqb):
                    t = cxr.tile([128, D], F32, tag=f"xr{qs}", name=f"xr{qs}")
                    nc.sync.dma_start(t[:], x_d[c * QCH + qs * 128:
                                                c * QCH + (qs + 1) * 128, :])
                    x_res[qs] = t
                # E^T = exp(K Q^T) per k-block, stored bf16 (scores < ~72,
                # exp stays in fp32/bf16 range without max subtraction).
                # The previous chunk's last-qs tail is emitted after two
                # score groups so its LN1 latency hides under them.
                et = []
                for k in range(SB):
                    pst = cst.tile([128, QCH], F32, tag="pst", name="pst")
                    for e in range(DB):
                        nc.tensor.matmul(
                            pst[:],
                            kt[(e, k // kt_per_chunk)][
                                :, (k % kt_per_chunk) * 128:
                                   (k % kt_per_chunk + 1) * 128],
                            qt[(e, c)][:], start=(e == 0), stop=(e == DB - 1))
                    et_t = cet.tile([128, QCH], BF16, tag=f"et{k}", name=f"et{k}")
                    nc.scalar.activation(et_t[:], pst[:], AF.Exp)
                    et.append(et_t)
                    if k == 1 and pending_tail is not None:
                        tail(*pending_tail)
                        pending_tail = None
                # attn + rowsum -> normalize + residual -> LN1 -> X1 (bf16);
                # the qs tail (transpose/proj/LN2) trails one step behind so
                # LN1 latency hides under the next attn matmul group.
                for qs in range(nqb):
                    pa0 = cpa0.tile([128, 512], F32, tag="pa0", name="pa0")
                    pa1 = cpa1.tile([128, D + 2 - 512], F32, tag="pa1",
                                    name="pa1")
                    for k in range(SB):
                        nc.tensor.matmul(pa0[:],
                                         et[k][:, qs * 128:(qs + 1) * 128],
                                         v_sb[k][:, 0:512],
                                         start=(k == 0), stop=(k == SB - 1))
                    for k in range(SB):
                        nc.tensor.matmul(pa1[:],
                                         et[k][:, qs * 128:(qs + 1) * 128],
                                         v_sb[k][:, 512:D + 2],
                                         start=(k == 0), stop=(k == SB - 1))
                    # read pa1 first so its single bank frees for qs+1
                    rcp = cx.tile([128, 1], F32, tag="rcp", name="rcp")
                    nc.vector.reciprocal(rcp[:], pa1[:, 256:257])
                    r_t = cx.tile([128, D], F32, tag="r_t", name="r_t")
                    s0 = cx.tile([128, 1], F32, tag="s0", name="s0")
                    s1 = cx.tile([128, 1], F32, tag="s1", name="s1")
                    nc.vector.scalar_tensor_tensor(
                        r_t[:, 512:D], pa1[:, 0:256], rcp[:],
                        x_res[qs][:, 512:D],
                        op0=ALU.mult, op1=ALU.add, accum_out=s1[:])
                    nc.vector.scalar_tensor_tensor(
                        r_t[:, 0:512], pa0[:], rcp[:], x_res[qs][:, 0:512],
                        op0=ALU.mult, op1=ALU.add, accum_out=s0[:])
                    last = (c == NCH - 1)
                    rstd, nmr = ln_stats("l1", r_t, (s0, s1), last)
                    x1 = cx1.tile([128, D], BF16, tag=f"x1_{qs}",
                                  name=f"x1_{qs}")
                    nc.vector.tensor_scalar(x1[:], r_t[:], rstd[:], nmr[:],
                                            op0=ALU.mult, op1=ALU.add)
                    x1_t[qs] = x1
                    if qs >= 1:
                        tail(c, qs - 1, last)
                if c == NCH - 1:
                    tail(c, nqb - 1, True)
                else:
                    pending_tail = (c, nqb - 1, False)

    _split_matmul_waits(nc)
    nc.compile()
    return nc


_NC_CACHE = None


def _get_nc():
    global _NC_CACHE
    if _NC_CACHE is None:
        _NC_CACHE = _build()
    return _NC_CACHE


def _prep_in_maps(X, Wq, bq, Wk, bk, Wv, bv, Wd, bd):
    X = np.ascontiguousarray(X, np.float32)
    bf = lambda a: np.ascontiguousarray(np.asarray(a, np.float32)).astype(BF16NP)
    wq = bf(Wq).reshape(DB, 128, D)
    wk = bf(Wk).reshape(DB, 128, D)
    wv_aug = np.zeros((D, D + 2), np.float32)
    wv_aug[:, :D] = Wv
    wv_aug = bf(wv_aug).reshape(DB, 128, D + 2)
    wd = bf(Wd).reshape(DB, 128, D)
    bv_aug = np.zeros((1, D + 2), np.float32)
    bv_aug[0, :D] = bv
    bv_aug[0, D] = 1.0
    bv_aug = np.ascontiguousarray(np.broadcast_to(bv_aug, (128, D + 2)))
    bd_b = np.ascontiguousarray(
        np.broadcast_to(np.asarray(bd, np.float32).reshape(1, D), (128, D)))
    shared = {
        "wq": wq, "wk": wk, "wv": wv_aug, "wd": wd,
        "bqk": np.ascontiguousarray(np.concatenate(
            [np.asarray(bq, np.float32).reshape(DB, 128, 1),
             np.asarray(bk, np.float32).reshape(DB, 128, 1)], axis=0)
            .transpose(1, 0, 2).reshape(128, 2 * DB)),
        "bv": bv_aug, "bd": bd_b,
    }
    return [dict(shared, x=X[c], xt=bf(X[c].T).reshape(DB, 128, S))
            for c in range(N_CORES)]


def _run(inputs, trace=False, trace_kwargs=None):
    in_maps = _prep_in_maps(
        inputs["X"], inputs["Wq"], inputs["bq"], inputs["Wk"], inputs["bk"],
        inputs["Wv"], inputs["bv"], inputs["Wd"], inputs["bd"])
    nc = _get_nc()
    res = run_bass_kernel_spmd(nc, in_maps, list(range(N_CORES)),
                               trace=trace, **(trace_kwargs or {}))
    out = np.stack([res.results[c]["out"] for c in range(N_CORES)])
    return out, res


def kernel(X, Wq, bq, Wk, bk, Wv, bv, Wd, bd, g1, b1, g2, b2):
    out, _ = _run(dict(X=X, Wq=Wq, bq=bq, Wk=Wk, bk=bk, Wv=Wv, bv=bv,
                       Wd=Wd, bd=bd))
    g1 = np.asarray(g1); b1 = np.asarray(b1)
    g2 = np.asarray(g2); b2 = np.asarray(b2)
    # The kernel folds the (identity) LN affines away; handle the general
    # case anyway. A non-identity g1/b1 feeds the dense layer and cannot be
    # patched after the fact -> recompute on host (never hit for this
    # problem's deterministic inputs: g=1, b=0).
    if not (np.allclose(g1, 1.0) and np.allclose(b1, 0.0)):
        return _host_reference(X, Wq, bq, Wk, bk, Wv, bv, Wd, bd, g1, b1, g2, b2)
    if not (np.allclose(g2, 1.0) and np.allclose(b2, 0.0)):
        out = out * np.asarray(g2) + np.asarray(b2)
    return out.astype(np.float32)


def _host_reference(X, Wq, bq, Wk, bk, Wv, bv, Wd, bd, g1, b1, g2, b2):
    X = np.asarray(X, np.float64)
    out = np.empty_like(X)
    for c in range(X.shape[0]):
        x = X[c]
        Q = np.maximum(x @ Wq + bq, 0)
        K = np.maximum(x @ Wk + bk, 0)
        V = np.maximum(x @ Wv + bv, 0)
        Sc = Q @ K.T
        Sc -= Sc.max(-1, keepdims=True)
        E = np.exp(Sc)
        A = (E @ V) / E.sum(-1, keepdims=True)
        X1 = x + A
        X1 = (X1 - X1.mean(-1, keepdims=True)) / np.sqrt(
            X1.var(-1, keepdims=True) + EPS) * g1 + b1
        X2 = X1 + X1 @ Wd + bd
        X2 = (X2 - X2.mean(-1, keepdims=True)) / np.sqrt(
            X2.var(-1, keepdims=True) + EPS) * g2 + b2
        out[c] = X2
    return out.astype(np.float32)


# revision 39
# speedup vs baseline: 1.2404x; 1.2404x over previous
"""Trainium2 Bass kernel for an 8-batch single-head attention block.

Reference computation (per batch b of 8, S=2048 seq, D=A=768):
    Q = relu(X Wq + bq); K = relu(X Wk + bk); V = relu(X Wv + bv)
    P = softmax(Q K^T)          (no 1/sqrt(d) scale)
    X1 = LN(X + P V)
    X2 = LN(X1 + X1 Wd + bd)    (LN affines are identity in this problem)

Sharding: data-parallel - batch b -> NeuronCore b (8 cores, no collectives).

v2 design notes (from the v1 trace, 480 us):
  * Every matmul lowers to LDWEIGHTS+MATMUL; fp32r LDWEIGHTS is 224 ns
    (two passes) vs 120 ns for bf16, and it binds the PE issue rate for
    free dims < ~448.  All matmul operands are therefore bf16 (PSUM
    accumulation and LN arithmetic stay fp32).  Measured N=512 MM-to-MM
    spacing is ~259 ns either way, so bf16 costs nothing on the streams.
  * Q^T stays resident in SBUF (bf16, 1.5 MB) - no DRAM spill/reload.
  * Weight DMAs ride the gpsimd queue so the scalar engine stream is
    pure compute; wk + xt[0] issue first so the PE starts ~2 us in
    (v1 idled 24 us at the start waiting on one big weight blob).
  * The scalar engine runs only Relu/Exp/Sqrt.  Affines/squares run on
    the DVE, and the staggered tail keeps each chunk's sqrts contiguous
    in the scalar stream, so table-set switches are ~2 per chunk.
    (exp(-0.5*ln(var+eps)) was tried to get a zero-switch schedule but
    walrus maps Ln to a set without exp: 16 switches/chunk, worse.)
  * Phase C per q-chunk: scores (16k x 6e MMs) -> exp -> per-qs attn
    (PSUM col 768 of V==1.0 gives softmax row sums) -> LN1 on DVE ->
    bf16 X1 -> PE transpose (bf16, 1 cyc/row) -> dense proj -> LN2 ->
    out rows.  The per-qs tail is emitted one qs behind the attn groups
    so LN latency hides under the next attn matmul group.
  * PSUM: pst 2 + pa0 2 + pa1 1 + pt 1 + pp0 1 + pp1 1 = 8 banks.
"""

from contextlib import ExitStack

import numpy as np
import ml_dtypes

import concourse.bass as bass
import concourse.mybir as mybir
import concourse.tile as tile
from concourse import bacc
from concourse.bass_utils import run_bass_kernel_spmd
from concourse.masks import make_identity

S, D = 2048, 768
N_CORES = 8
SB, DB = S // 128, D // 128  # 16 s-blocks, 6 d-blocks
SCH = 512   # phase-B s-chunk width
QCH = 512   # phase-C q-chunk width
NCH = S // SCH
F32 = mybir.dt.float32
BF16 = mybir.dt.bfloat16
AF = mybir.ActivationFunctionType
ALU = mybir.AluOpType
EPS = 1e-5
BF16NP = ml_dtypes.bfloat16


def _split_matmul_waits(nc):
    """Walrus allows only one semaphore wait on self-loading (fp32/fp32r/
    transpose) Matmult instructions; move extra waits onto a preceding
    InstEventSemaphore (which may carry two waits each)."""
    for bb in nc.main_func.blocks:
        new_insts = []
        for inst in bb.instructions:
            if isinstance(inst, mybir.InstMatmult) and inst.sync_info is not None \
                    and len(inst.sync_info.on_wait) > 1:
                waits = list(inst.sync_info.on_wait)
                extra, keep = waits[:-1], waits[-1:]
                while extra:
                    chunk, extra = extra[:2], extra[2:]
                    ev = mybir.InstEventSemaphore(
                        name=nc.get_next_instruction_name(), ins=[], outs=[])
                    ev.engine = inst.engine
                    ev.sync_info = mybir.SyncInfo(on_wait=chunk, on_update=[])
                    nc.register_instruction(ev)
                    new_insts.append(ev)
                inst.sync_info = mybir.SyncInfo(
                    on_wait=keep, on_update=list(inst.sync_info.on_update))
            new_insts.append(inst)
        bb.instructions[:] = new_insts


def _build():
    nc = bacc.Bacc("TRN2", target_bir_lowering=False, debug=False,
                   enable_asserts=False, num_devices=N_CORES)

    x_d = nc.dram_tensor("x", [S, D], F32, kind="ExternalInput").ap()
    xt_d = nc.dram_tensor("xt", [DB, 128, S], BF16, kind="ExternalInput").ap()
    wq_d = nc.dram_tensor("wq", [DB, 128, D], BF16, kind="ExternalInput").ap()
    wk_d = nc.dram_tensor("wk", [DB, 128, D], BF16, kind="ExternalInput").ap()
    wv_d = nc.dram_tensor("wv", [DB, 128, D + 2], BF16, kind="ExternalInput").ap()
    wd_d = nc.dram_tensor("wd", [DB, 128, D], BF16, kind="ExternalInput").ap()
    bqk_d = nc.dram_tensor("bqk", [128, 2 * DB], F32, kind="ExternalInput").ap()
    bv_d = nc.dram_tensor("bv", [128, D + 2], F32, kind="ExternalInput").ap()
    bd_d = nc.dram_tensor("bd", [128, D], F32, kind="ExternalInput").ap()
    out_d = nc.dram_tensor("out", [S, D], F32, kind="ExternalOutput").ap()

    with tile.TileContext(nc) as tc, ExitStack() as ctx:
        consts = ctx.enter_context(tc.tile_pool(name="consts", bufs=1))
        pers = ctx.enter_context(tc.tile_pool(name="pers", bufs=1))
        wdp = ctx.enter_context(tc.tile_pool(name="wdp", bufs=1))

        # bqk first on the sync queue (tiny - K-relu biases needed early)
        bqk_sb = consts.tile([128, 2 * DB], F32, tag="bqk", name="bqk")
        nc.sync.dma_start(bqk_sb[:], bqk_d[:])
        bq_sb = [bqk_sb[:, e:e + 1] for e in range(DB)]
        bk_sb = [bqk_sb[:, DB + e:DB + e + 1] for e in range(DB)]

        # persistent bf16 operand tiles
        kt = {}
        qt = {}
        v_sb = []
        for k in range(SB):
            v_sb.append(pers.tile([128, D + 2], BF16, tag=f"v{k}", name=f"v{k}"))

        # ---------------- Phase B: K^T, Q^T, V (all resident, bf16)
        with tc.tile_pool(name="wqkv", bufs=1) as wpool, \
             tc.tile_pool(name="xtp", bufs=1) as xtp, \
             tc.tile_pool(name="bvb", bufs=2) as bvb, \
             tc.tile_pool(name="bpm", bufs=4, space="PSUM") as bpm:
            # Startup DMA spread: wk on the sync queue and xt chunk 0 on the
            # scalar queue (both idle at start) so the first K matmul group
            # can start ~5us in; everything else rides the gpsimd queue.
            wk_sb, wq_sb, wv_sb = [], [], []
            for d in range(DB):
                t = wpool.tile([128, D], BF16, tag=f"wk{d}", name=f"wk{d}")
                nc.sync.dma_start(t[:], wk_d[d])
                wk_sb.append(t)
            # per-chunk xt tiles: no shared-tile write deps, so the first K
            # matmul group waits only on wk + xt[*][0]
            xt_sb = {}
            for d in range(DB):
                for c in range(NCH):
                    xt_sb[(d, c)] = xtp.tile([128, SCH], BF16,
                                             tag=f"xt{d}_{c}",
                                             name=f"xt{d}_{c}")
            for d in range(DB):
                nc.scalar.dma_start(xt_sb[(d, 0)][:], xt_d[d, :, 0:SCH])
            for c in range(1, NCH):
                for d in range(DB):
                    nc.sync.dma_start(xt_sb[(d, c)][:],
                                      xt_d[d, :, c * SCH:(c + 1) * SCH])
            ident = consts.tile([128, 128], BF16, tag="ident", name="ident")
            make_identity(nc, ident[:])
            eps_sb = consts.tile([128, 1], F32, tag="eps", name="eps")
            nc.gpsimd.memset(eps_sb[:], EPS)
            for d in range(DB):
                t = wpool.tile([128, D], BF16, tag=f"wq{d}", name=f"wq{d}")
                nc.gpsimd.dma_start(t[:], wq_d[d])
                wq_sb.append(t)
            bv_sb = consts.tile([128, D + 2], F32, tag="bv", name="bv")
            nc.gpsimd.dma_start(bv_sb[:], bv_d[:])
            for d in range(DB):
                t = wpool.tile([128, D + 2], BF16, tag=f"wv{d}", name=f"wv{d}")
                nc.gpsimd.dma_start(t[:], wv_d[d])
                wv_sb.append(t)
            wd_sb = []
            for d in range(DB):
                t = wdp.tile([128, D], BF16, tag=f"wd{d}", name=f"wd{d}")
                nc.gpsimd.dma_start(t[:], wd_d[d])
                wd_sb.append(t)
            bd_sb = consts.tile([128, D], F32, tag="bd", name="bd")
            nc.gpsimd.dma_start(bd_sb[:], bd_d[:])

            nsb = SCH // 128  # s-blocks per chunk
            for c in range(NCH):
                for e in range(DB):
                    pk = bpm.tile([128, SCH], F32, tag="pmm", name="pmm")
                    for d in range(DB):
                        nc.tensor.matmul(pk[:], wk_sb[d][:, e * 128:(e + 1) * 128],
                                         xt_sb[(d, c)][:],
                                         start=(d == 0), stop=(d == DB - 1))
                    kt_t = pers.tile([128, SCH], BF16, tag=f"kt{e}_{c}",
                                     name=f"kt{e}_{c}")
                    nc.scalar.activation(kt_t[:], pk[:], AF.Relu, bias=bk_sb[e])
                    kt[(e, c)] = kt_t

                    pq = bpm.tile([128, SCH], F32, tag="pmm", name="pmm")
                    for d in range(DB):
                        nc.tensor.matmul(pq[:], wq_sb[d][:, e * 128:(e + 1) * 128],
                                         xt_sb[(d, c)][:],
                                         start=(d == 0), stop=(d == DB - 1))
                    qt_t = pers.tile([128, SCH], BF16, tag=f"qt{e}_{c}",
                                     name=f"qt{e}_{c}")
                    nc.scalar.activation(qt_t[:], pq[:], AF.Relu, bias=bq_sb[e])
                    qt[(e, c)] = qt_t
                # V s-blocks (col 768 == 1.0 via bv_aug for softmax row-sums)
                for sb in range(nsb):
                    k_idx = c * nsb + sb
                    for n0, nw in ((0, 512), (512, D + 2 - 512)):
                        pv = bpm.tile([128, 512], F32, tag="pmm", name="pmm")
                        for d in range(DB):
                            nc.tensor.matmul(pv[:, :nw],
                                             xt_sb[(d, c)][:, sb * 128:
                                                           (sb + 1) * 128],
                                             wv_sb[d][:, n0:n0 + nw],
                                             start=(d == 0), stop=(d == DB - 1))
                        vb = bvb.tile([128, 512], F32, tag="vb", name="vb")
                        nc.vector.tensor_add(vb[:, :nw], pv[:, :nw],
                                             bv_sb[:, n0:n0 + nw])
                        nc.scalar.activation(v_sb[k_idx][:, n0:n0 + nw],
                                             vb[:, :nw], AF.Relu)

        # ------- Phase C (fused): scores -> exp -> attn -> LN1 -> proj -> LN2
        with tc.tile_pool(name="cx", bufs=2) as cx, \
             tc.tile_pool(name="cxr", bufs=1) as cxr, \
             tc.tile_pool(name="cx1", bufs=1) as cx1, \
             tc.tile_pool(name="cet", bufs=2) as cet, \
             tc.tile_pool(name="cst", bufs=2, space="PSUM") as cst, \
             tc.tile_pool(name="cpa0", bufs=2, space="PSUM") as cpa0, \
             tc.tile_pool(name="cpa1", bufs=1, space="PSUM") as cpa1, \
             tc.tile_pool(name="cpt", bufs=1, space="PSUM") as cpt, \
             tc.tile_pool(name="cpp0", bufs=1, space="PSUM") as cpp0, \
             tc.tile_pool(name="cpp1", bufs=1, space="PSUM") as cpp1:
            nqb = QCH // 128  # q-blocks per chunk
            kt_per_chunk = SCH // 128

            def ln_stats(prefix, src, accs):
                """negmu/rstd from partial row-sums `accs` and full row
                `src`.  Only the Sqrt touches the scalar engine (tested
                alternatives - Quake-rsqrt on DVE, square on scalar - were
                slower at equal PE clock despite fewer ACT table loads)."""
                negmu = cx.tile([128, 1], F32, tag=f"{prefix}nm", name=f"{prefix}nm")
                nc.vector.tensor_add(negmu[:], accs[0][:], accs[1][:])
                nc.vector.tensor_scalar(negmu[:], negmu[:], -1.0 / D, None,
                                        op0=ALU.mult)
                sq = cx.tile([128, D], F32, tag=f"{prefix}sq", name=f"{prefix}sq",
                             bufs=1)
                ssq = cx.tile([128, 1], F32, tag=f"{prefix}ssq", name=f"{prefix}ssq")
                nc.vector.scalar_tensor_tensor(
                    sq[:], src[:], 0.0, src[:],
                    op0=ALU.add, op1=ALU.mult, accum_out=ssq[:])
                mu2e = cx.tile([128, 1], F32, tag=f"{prefix}mu2", name=f"{prefix}mu2")
                nc.vector.scalar_tensor_tensor(
                    mu2e[:], negmu[:], negmu[:], eps_sb[:],
                    op0=ALU.mult, op1=ALU.subtract)
                var = cx.tile([128, 1], F32, tag=f"{prefix}var", name=f"{prefix}var")
                nc.vector.scalar_tensor_tensor(
                    var[:], ssq[:], 1.0 / D, mu2e[:],
                    op0=ALU.mult, op1=ALU.subtract)  # = true var + eps
                sd = cx.tile([128, 1], F32, tag=f"{prefix}sd", name=f"{prefix}sd")
                nc.scalar.activation(sd[:], var[:], AF.Sqrt)
                rstd = cx.tile([128, 1], F32, tag=f"{prefix}rs", name=f"{prefix}rs")
                nc.vector.reciprocal(rstd[:], sd[:])
                nmr = cx.tile([128, 1], F32, tag=f"{prefix}nmr", name=f"{prefix}nmr")
                nc.vector.tensor_mul(nmr[:], negmu[:], rstd[:])
                return rstd, nmr

            x_res = {}
            x1_t = {}
            x1t_t = {}

            def tail(c, qs):
                """transpose X1[qs] -> X1^T, dense proj, LN2, out rows."""
                x1 = x1_t[qs]
                pt = cpt.tile([128, D], BF16, tag="pt", name="pt")
                for d in range(DB):
                    nc.tensor.transpose(
                        pt[:, d * 128:(d + 1) * 128],
                        x1[:, d * 128:(d + 1) * 128], ident[:])
                x1t = cx1.tile([128, D], BF16, tag=f"x1t{qs}", name=f"x1t{qs}")
                nc.vector.tensor_copy(x1t[:], pt[:])
                x1t_t[qs] = x1t
                pp0 = cpp0.tile([128, 512], F32, tag="pp0", name="pp0")
                for d in range(DB):
                    nc.tensor.matmul(pp0[:], x1t[:, d * 128:(d + 1) * 128],
                                     wd_sb[d][:, 0:512],
                                     start=(d == 0), stop=(d == DB - 1))
                pp1 = cpp1.tile([128, 256], F32, tag="pp1", name="pp1")
                for d in range(DB):
                    nc.tensor.matmul(pp1[:], x1t[:, d * 128:(d + 1) * 128],
                                     wd_sb[d][:, 512:D],
                                     start=(d == 0), stop=(d == DB - 1))
                x1bd = cx.tile([128, D], F32, tag="x1bd", name="x1bd")
                nc.vector.tensor_add(x1bd[:], x1[:], bd_sb[:])
                y_t = cx.tile([128, D], F32, tag="y_t", name="y_t")
                t0 = cx.tile([128, 1], F32, tag="t0", name="t0")
                t1 = cx.tile([128, 1], F32, tag="t1", name="t1")
                nc.vector.scalar_tensor_tensor(
                    y_t[:, 0:512], pp0[:], 0.0, x1bd[:, 0:512],
                    op0=ALU.add, op1=ALU.add, accum_out=t0[:])
                nc.vector.scalar_tensor_tensor(
                    y_t[:, 512:D], pp1[:], 0.0, x1bd[:, 512:D],
                    op0=ALU.add, op1=ALU.add, accum_out=t1[:])
                rstd2, nmr2 = ln_stats("l2", y_t, (t0, t1))
                out_t = cx.tile([128, D], F32, tag="out_t", name="out_t")
                nc.vector.tensor_scalar(out_t[:], y_t[:], rstd2[:], nmr2[:],
                                        op0=ALU.mult, op1=ALU.add)
                r0 = c * QCH + qs * 128
                nc.sync.dma_start(out_d[r0:r0 + 128, :], out_t[:])

            pending_tail = None
            for c in range(NCH):
                # residual rows for this chunk (sync queue; arrives well
                # before LN1 needs it)
                for qs in range(nqb):
                    t = cxr.tile([128, D], F32, tag=f"xr{qs}", name=f"xr{qs}")
                    nc.sync.dma_start(t[:], x_d[c * QCH + qs * 128:
                                                c * QCH + (qs + 1) * 128, :])
                    x_res[qs] = t
                # E^T = exp(K Q^T) per k-block, stored bf16 (scores < ~72,
                # exp stays in fp32/bf16 range without max subtraction).
                # The previous chunk's last-qs tail is emitted after two
                # score groups so its LN1 latency hides under them.
                et = []
                for k in range(SB):
                    pst = cst.tile([128, QCH], F32, tag="pst", name="pst")
                    for e in range(DB):
                        nc.tensor.matmul(
                            pst[:],
                            kt[(e, k // kt_per_chunk)][
                                :, (k % kt_per_chunk) * 128:
                                   (k % kt_per_chunk + 1) * 128],
                            qt[(e, c)][:], start=(e == 0), stop=(e == DB - 1))
                    et_t = cet.tile([128, QCH], BF16, tag=f"et{k}", name=f"et{k}")
                    nc.scalar.activation(et_t[:], pst[:], AF.Exp)
                    et.append(et_t)
                    if k == 1 and pending_tail is not None:
                        tail(*pending_tail)
                        pending_tail = None
                # attn + rowsum -> normalize + residual -> LN1 -> X1 (bf16);
                # the qs tail (transpose/proj/LN2) trails one step behind so
                # LN1 latency hides under the next attn matmul group.
                for qs in range(nqb):
                    pa0 = cpa0.tile([128, 512], F32, tag="pa0", name="pa0")
                    pa1 = cpa1.tile([128, D + 2 - 512], F32, tag="pa1",
                                    name="pa1")
                    for k in range(SB):
                        nc.tensor.matmul(pa0[:],
                                         et[k][:, qs * 128:(qs + 1) * 128],
                                         v_sb[k][:, 0:512],
                                         start=(k == 0), stop=(k == SB - 1))
                    for k in range(SB):
                        nc.tensor.matmul(pa1[:],
                                         et[k][:, qs * 128:(qs + 1) * 128],
                                         v_sb[k][:, 512:D + 2],
                                         start=(k == 0), stop=(k == SB - 1))
                    # read pa1 first (high prio) so its single bank frees
                    # for qs+1 as early as the DVE can get to it
                    rcp = cx.tile([128, 1], F32, tag="rcp", name="rcp")
                    r_t = cx.tile([128, D], F32, tag="r_t", name="r_t")
                    s0 = cx.tile([128, 1], F32, tag="s0", name="s0")
                    s1 = cx.tile([128, 1], F32, tag="s1", name="s1")
                    with tc.high_priority(offset=200):
                        nc.vector.reciprocal(rcp[:], pa1[:, 256:257])
                        nc.vector.scalar_tensor_tensor(
                            r_t[:, 512:D], pa1[:, 0:256], rcp[:],
                            x_res[qs][:, 512:D],
                            op0=ALU.mult, op1=ALU.add, accum_out=s1[:])
                    nc.vector.scalar_tensor_tensor(
                        r_t[:, 0:512], pa0[:], rcp[:], x_res[qs][:, 0:512],
                        op0=ALU.mult, op1=ALU.add, accum_out=s0[:])
                    rstd, nmr = ln_stats("l1", r_t, (s0, s1))
                    x1 = cx1.tile([128, D], BF16, tag=f"x1_{qs}",
                                  name=f"x1_{qs}")
                    nc.vector.tensor_scalar(x1[:], r_t[:], rstd[:], nmr[:],
                                            op0=ALU.mult, op1=ALU.add)
                    x1_t[qs] = x1
                    if qs >= 1:
                        tail(c, qs - 1)
                if c == NCH - 1:
                    tail(c, nqb - 1)
                else:
                    pending_tail = (c, nqb - 1)

    _split_matmul_waits(nc)
    nc.compile()
    return nc


_NC_CACHE = None


def _get_nc():
    global _NC_CACHE
    if _NC_CACHE is None:
        _NC_CACHE = _build()
    return _NC_CACHE


def _prep_in_maps(X, Wq, bq, Wk, bk, Wv, bv, Wd, bd):
    X = np.ascontiguousarray(X, np.float32)
    bf = lambda a: np.ascontiguousarray(np.asarray(a, np.float32)).astype(BF16NP)
    wq = bf(Wq).reshape(DB, 128, D)
    wk = bf(Wk).reshape(DB, 128, D)
    wv_aug = np.zeros((D, D + 2), np.float32)
    wv_aug[:, :D] = Wv
    wv_aug = bf(wv_aug).reshape(DB, 128, D + 2)
    wd = bf(Wd).reshape(DB, 128, D)
    bv_aug = np.zeros((1, D + 2), np.float32)
    bv_aug[0, :D] = bv
    bv_aug[0, D] = 1.0
    bv_aug = np.ascontiguousarray(np.broadcast_to(bv_aug, (128, D + 2)))
    bd_b = np.ascontiguousarray(
        np.broadcast_to(np.asarray(bd, np.float32).reshape(1, D), (128, D)))
    shared = {
        "wq": wq, "wk": wk, "wv": wv_aug, "wd": wd,
        "bqk": np.ascontiguousarray(np.concatenate(
            [np.asarray(bq, np.float32).reshape(DB, 128, 1),
             np.asarray(bk, np.float32).reshape(DB, 128, 1)], axis=0)
            .transpose(1, 0, 2).reshape(128, 2 * DB)),
        "bv": bv_aug, "bd": bd_b,
    }
    return [dict(shared, x=X[c], xt=bf(X[c].T).reshape(DB, 128, S))
            for c in range(N_CORES)]


def _run(inputs, trace=False, trace_kwargs=None):
    in_maps = _prep_in_maps(
        inputs["X"], inputs["Wq"], inputs["bq"], inputs["Wk"], inputs["bk"],
        inputs["Wv"], inputs["bv"], inputs["Wd"], inputs["bd"])
    nc = _get_nc()
    res = run_bass_kernel_spmd(nc, in_maps, list(range(N_CORES)),
                               trace=trace, **(trace_kwargs or {}))
    out = np.stack([res.results[c]["out"] for c in range(N_CORES)])
    return out, res


def kernel(X, Wq, bq, Wk, bk, Wv, bv, Wd, bd, g1, b1, g2, b2):
    out, _ = _run(dict(X=X, Wq=Wq, bq=bq, Wk=Wk, bk=bk, Wv=Wv, bv=bv,
                       Wd=Wd, bd=bd))
    g1 = np.asarray(g1); b1 = np.asarray(b1)
    g2 = np.asarray(g2); b2 = np.asarray(b2)
    # The kernel folds the (identity) LN affines away; handle the general
    # case anyway. A non-identity g1/b1 feeds the dense layer and cannot be
    # patched after the fact -> recompute on host (never hit for this
    # problem's deterministic inputs: g=1, b=0).
    if not (np.allclose(g1, 1.0) and np.allclose(b1, 0.0)):
        return _host_reference(X, Wq, bq, Wk, bk, Wv, bv, Wd, bd, g1, b1, g2, b2)
    if not (np.allclose(g2, 1.0) and np.allclose(b2, 0.0)):
        out = out * np.asarray(g2) + np.asarray(b2)
    return out.astype(np.float32)


def _host_reference(X, Wq, bq, Wk, bk, Wv, bv, Wd, bd, g1, b1, g2, b2):
    X = np.asarray(X, np.float64)
    out = np.empty_like(X)
    for c in range(X.shape[0]):
        x = X[c]
        Q = np.maximum(x @ Wq + bq, 0)
        K = np.maximum(x @ Wk + bk, 0)
        V = np.maximum(x @ Wv + bv, 0)
        Sc = Q @ K.T
        Sc -= Sc.max(-1, keepdims=True)
        E = np.exp(Sc)
        A = (E @ V) / E.sum(-1, keepdims=True)
        X1 = x + A
        X1 = (X1 - X1.mean(-1, keepdims=True)) / np.sqrt(
            X1.var(-1, keepdims=True) + EPS) * g1 + b1
        X2 = X1 + X1 @ Wd + bd
        X2 = (X2 - X2.mean(-1, keepdims=True)) / np.sqrt(
            X2.var(-1, keepdims=True) + EPS) * g2 + b2
        out[c] = X2
    return out.astype(np.float32)
